# revision 3
# baseline (speedup 1.0000x reference)
"""BitLinear (ternary-quantized linear) Trainium2 kernel, v2.

Computes: out = x @ dequant(weight).T where dequant is per-group(128)
AbsMean ternary quantization (w_q in {-1,0,+1} times per-group scale).

Strategy (8 NeuronCores, column-parallel / tensor-parallel):
  - weight [O=11008, K=4096] is sharded by rows across 8 cores (1376 each).
  - x [B,S,K] -> [T=8192, K] is replicated to every core, pre-packed and
    pre-cast to fp16 on host (same RNE rounding the on-chip ACT cast did)
    so the contraction dim lands on SBUF partitions and x DMA is halved.
  - The weight shard is uploaded pre-transposed to [k, o] blocks, together
    with the per-group thresholds s/2 (f32, partition-replicated).  The
    on-chip dequant is then purely elementwise (no PE transposes, no
    reductions): a = (w > s/2), b = (w < -s/2), q = a-b, wb = q*fp16(s),
    written straight into the SBUF-resident fp16 weight wbt[k, o].
    f32 thresholds keep ternary rounding decisions exact vs the reference
    (fp16 thresholds measured at 1.1% rel err from flipped borderlines).
  - x streams in t-tiles of 128, accumulating in PSUM over 32 k-groups,
    3 output chunks (512/512/352) per core.
  - Prefix pipelining: dequant is emitted chunk-major in 2-group blocks;
    the first W1 t-tiles run group-outer in a 4-wide window on chunk 0,
    pacing the PE behind the dequant pipeline; the next W2 run free on
    chunk 0; a few "mid" tiles run chunks 0-1; the rest run all chunks.
    Skipped chunks are caught up at the end.  Per-tile PE cost is
    identical either way, so the reorder costs nothing.
  - Per-core output [T, 1376] (t-major); host concatenates along O.
"""

import os

import numpy as np

import concourse.bass as bass
import concourse.mybir as mybir
import concourse.tile as tile
from concourse import bacc
from concourse.bass_utils import run_bass_kernel_spmd

P = 128
GROUP = 128
EPS = 1e-8

# Full problem shapes (hardcoded; harness calls kernel() with these).
FULL_B, FULL_S, FULL_K, FULL_O = 4, 2048, 4096, 11008
N_CORES = 8

DQB = 2          # k-groups per dequant block
CPAD = 512       # per-group chunk width in the padded wt/tp upload
W1N = 4          # paced warmup tiles (chunk 0, group-outer window)
W2N = 4          # free warmup tiles (chunk 0)
MIDN = 6         # tiles running chunks 0-1 only

LAST_RESULT = None  # BassKernelResults of the most recent run (for test.py)


def build_program(K, T, O_SHARD, mm_dt=mybir.dt.float16):
    """One SPMD program, identical on every core (data differs per core).

    DRAM tensors:
      xt  [T, K] fp16 ExternalInput -- x pre-packed on host (see pack_x) so
          that the per-t-tile load xt[tt*P+p, ko*G+t] = x[tt*P+t, ko*G+p]
          is one fully contiguous 1MB block (8KB per partition row)
      wt  [3, KO//DQB, P, DQB*CPAD] f32 ExternalInput -- weight shard,
          transposed to [k, o] and blocked per (chunk, group-pair); the
          352-wide chunk is zero-padded to CPAD
      tp  same shape/layout f32 -- thresholds s/2, replicated across the
          128 partitions (all k rows of a group share the group scale)
      out [T, O_SHARD] f32 ExternalOutput
    """
    assert K % GROUP == 0 and T % P == 0
    KO = K // GROUP
    NB = KO // DQB  # dequant blocks per chunk
    n_ttiles = T // P
    OC = 512
    chunks = [(c0, min(OC, O_SHARD - c0)) for c0 in range(0, O_SHARD, OC)]
    n_chunks = len(chunks)
    W = DQB * CPAD

    nc = bacc.Bacc("TRN2", target_bir_lowering=False, debug=False)
    xt = nc.dram_tensor("xt", [T, K], mm_dt, kind="ExternalInput").ap()
    wt = nc.dram_tensor(
        "wt", [n_chunks, NB, P, W], mybir.dt.float32, kind="ExternalInput"
    ).ap()
    tp = nc.dram_tensor(
        "tp", [n_chunks, NB, P, W], mybir.dt.float32, kind="ExternalInput"
    ).ap()
    out = nc.dram_tensor(
        "out", [T, O_SHARD], mybir.dt.float32, kind="ExternalOutput"
    ).ap()

    with tile.TileContext(nc) as tc:
        with (
            tc.tile_pool(name="wres", bufs=1) as wres,
            tc.tile_pool(name="wload", bufs=2) as wload,
            tc.tile_pool(name="tpload", bufs=2) as tpload,
            tc.tile_pool(name="deq", bufs=2) as deq,
            tc.tile_pool(name="xin", bufs=5) as xin,
            tc.tile_pool(name="outp", bufs=2) as outp,
            tc.tile_pool(name="ps_c0", bufs=4, space="PSUM") as ps_c0,
            tc.tile_pool(name="ps_12", bufs=2, space="PSUM") as ps_12,
        ):
            # Resident dequantized weight, [k-part, group, o], one per chunk.
            wbt = [wres.tile([P, KO, csz], mm_dt, tag=f"wbt{ci}", name=f"wbt{ci}")
                   for ci, (c0, csz) in enumerate(chunks)]

            # ---------------- dequant of one (chunk, group-pair) ----------
            def dequant_block(ci, qb):
                c0, csz = chunks[ci]
                g0 = qb * DQB
                # w/tp ride separate DMA rings (ACT + GPSIMD) so they never
                # queue behind the much larger x stream on the SP ring.
                wt_t = wload.tile([P, W], mybir.dt.float32, tag="wt")
                nc.scalar.dma_start(wt_t, wt[ci, qb])
                tp_t = tpload.tile([P, W], mybir.dt.float32, tag="tp")
                nc.gpsimd.dma_start(tp_t, tp[ci, qb])
                # s16 = fp16(2*(s/2)) == fp16(s) exactly (binary scaling)
                s16 = deq.tile([P, W], mm_dt, tag="s16")
                nc.scalar.mul(s16, tp_t, 2.0)
                a = deq.tile([P, W], mm_dt, tag="a")
                nc.vector.tensor_tensor(a, wt_t, tp_t, mybir.AluOpType.is_gt)
                # tneg = -tpos, in place once the is_gt above has consumed it
                nc.scalar.mul(tp_t, tp_t, -1.0)
                b = deq.tile([P, W], mm_dt, tag="b")
                nc.vector.tensor_tensor(b, wt_t, tp_t, mybir.AluOpType.is_lt)
                # q = a - b in {-1,0,1}; wb = q * fp16(s) (exact products)
                # (Pool/GpSimd only supports integer add/mult/sub - no cmp)
                nc.gpsimd.tensor_tensor(a, a, b, mybir.AluOpType.subtract)
                a3 = a.rearrange("p (g c) -> p g c", c=CPAD)
                s3 = s16.rearrange("p (g c) -> p g c", c=CPAD)
                nc.vector.tensor_tensor(
                    wbt[ci][:, g0 : g0 + DQB, :],
                    a3[:, :, :csz],
                    s3[:, :, :csz],
                    mybir.AluOpType.mult,
                )

            # ---------------- matmul windows ----------------
            # host-packed: xt_r[tt, p, ko, t] = x[tt*P + t, ko*G + p]
            xt_r = xt.rearrange("(tt p) (ko t) -> tt p ko t", p=P, t=P)

            def mm_tiles(tts, cis):
                """Process t-tiles `tts` in lockstep (group-outer) over
                output chunks `cis`; len(tts) * len(cis) PSUM banks live."""
                xbs, pss = {}, {}
                for tt in tts:
                    xb = xin.tile([P, KO, P], mm_dt, tag="xb", name=f"xb{tt}")
                    nc.sync.dma_start(xb, xt_r[tt])
                    xbs[tt] = xb
                    for ci in cis:
                        pool = ps_c0 if ci == 0 else ps_12
                        ps = pool.tile([P, chunks[ci][1]], mybir.dt.float32,
                                       tag=f"mm{ci}", name=f"mm{ci}_{tt}")
                        pss[(tt, ci)] = ps
                for ko in range(KO):
                    for tt in tts:
                        for ci in cis:
                            nc.tensor.matmul(
                                pss[(tt, ci)],
                                lhsT=xbs[tt][:, ko, :],
                                rhs=wbt[ci][:, ko, :],
                                start=(ko == 0),
                                stop=(ko == KO - 1),
                            )
                w0 = chunks[cis[0]][0]
                wid = sum(chunks[ci][1] for ci in cis)
                for tt in tts:
                    ot = outp.tile([P, O_SHARD], mybir.dt.float32, tag="ot",
                                   name="ot")[:, :wid]
                    for ci in cis:
                        c0, csz = chunks[ci]
                        nc.scalar.copy(ot[:, c0 - w0 : c0 - w0 + csz],
                                       pss[(tt, ci)])
                    nc.sync.dma_start(
                        out[tt * P : tt * P + P, w0 : w0 + wid], ot
                    )

            # ---------------- emission order ----------------
            for ci in range(n_chunks):
                for qb in range(NB):
                    dequant_block(ci, qb)

            all_cis = list(range(n_chunks))
            warm = list(range(W1N + W2N))
            mids = list(range(len(warm), len(warm) + MIDN))
            mm_tiles(warm[:W1N], [0])          # paced behind chunk-0 dequant
            mm_tiles(warm[W1N:], [0])          # runs free
            for tt in mids:
                mm_tiles([tt], [0, 1])         # chunk 2 still dequantizing
            for tt in range(len(warm) + MIDN, n_ttiles):
                mm_tiles([tt], all_cis)        # main loop
            for tt in warm:                    # catch-up
                mm_tiles([tt], all_cis[1:])
            for tt in mids:
                mm_tiles([tt], all_cis[2:])

    nc.compile()
    return nc


def _run(nc, in_maps, trace=False):
    global LAST_RESULT
    res = run_bass_kernel_spmd(
        nc, in_maps, core_ids=list(range(len(in_maps))), trace=trace
    )
    LAST_RESULT = res
    return res


def pack_x(x2d):
    """[T, K] f32 -> fp16 packed: H[tt*P+p, ko*G+t] = x2d[tt*P+t, ko*G+p]."""
    T, K = x2d.shape
    x4 = x2d.reshape(T // P, P, K // GROUP, GROUP)  # [tt, t, ko, p]
    return np.ascontiguousarray(
        x4.transpose(0, 3, 2, 1).reshape(T, K).astype(np.float16)
    )


def pack_w(w_shard, chunks):
    """Weight shard [O_S, K] f32 -> (wt, tp) blocked uploads.

    wt[ci, qb, p, gi*CPAD + u] = w_shard[c0 + u, (qb*DQB + gi)*GROUP + p]
    tp[ci, qb, p, gi*CPAD + u] = s[c0 + u, qb*DQB + gi] / 2
    (u < csz; zero-padded to CPAD).  s is the reference AbsMean scale.
    """
    O_S, K = w_shard.shape
    KO = K // GROUP
    NB = KO // DQB
    flat = w_shard.reshape(-1, GROUP).astype(np.float32)
    s = np.maximum(
        np.abs(flat).mean(axis=1, dtype=np.float32), np.float32(EPS)
    ).reshape(O_S, KO)                                    # [o, g]
    wT = np.ascontiguousarray(w_shard.T).reshape(KO, GROUP, O_S)  # [g, p, o]
    n_chunks = len(chunks)
    wt = np.zeros((n_chunks, NB, P, DQB * CPAD), dtype=np.float32)
    tp = np.zeros_like(wt)
    for ci, (c0, csz) in enumerate(chunks):
        blk = wT[:, :, c0 : c0 + csz].reshape(NB, DQB, P, csz)
        wt[ci, :, :, : DQB * CPAD].reshape(NB, P, DQB, CPAD)[
            ..., :csz
        ] = blk.transpose(0, 2, 1, 3)
        sblk = (s.T[:, c0 : c0 + csz] * np.float32(0.5)).reshape(NB, DQB, csz)
        tp[ci].reshape(NB, P, DQB, CPAD)[..., :csz] = sblk[:, None, :, :]
    return wt, tp


def kernel(x, weight):
    T = FULL_B * FULL_S
    K = FULL_K
    OS = FULL_O // N_CORES  # 1376
    chunks = [(c0, min(512, OS - c0)) for c0 in range(0, OS, 512)]
    x2d = pack_x(np.asarray(x, dtype=np.float32).reshape(T, K))
    w = np.asarray(weight, dtype=np.float32)

    nc = build_program(K, T, OS)
    in_maps = []
    for c in range(N_CORES):
        wt, tp = pack_w(w[c * OS : (c + 1) * OS], chunks)
        in_maps.append({"xt": x2d, "wt": wt, "tp": tp})
    trace = bool(os.environ.get("BASS_TRACE"))
    res = _run(nc, in_maps, trace=trace)
    full = np.concatenate(
        [res.results[c]["out"] for c in range(N_CORES)], axis=1
    )
    return np.ascontiguousarray(full.reshape(FULL_B, FULL_S, FULL_O))


# revision 4
# speedup vs baseline: 1.1502x; 1.1502x over previous
"""BitLinear (ternary-quantized linear) Trainium2 kernel, v3.

Computes: out = x @ dequant(weight).T where dequant is per-group(128)
AbsMean ternary quantization (w_q in {-1,0,+1} times per-group scale).

Strategy (8 NeuronCores, column-parallel / tensor-parallel):
  - weight [O=11008, K=4096] sharded by rows across 8 cores (1376 each).
  - x [B,S,K] -> [T=8192, K] replicated, pre-packed + pre-cast to fp16 on
    host (same RNE rounding the on-chip ACT cast would do) so the
    contraction dim lands on SBUF partitions and x DMA is halved.
  - Weight shard uploaded pre-transposed to [k, o] blocks with the
    per-group thresholds s/2 (f32, partition-replicated) and scales
    fp16(s).  On-chip dequant is purely elementwise, no PE involvement:
      sg = Sign(w)            (ACT)
      aw = |w|     (in-place) (ACT)
      c  = aw > s/2           (DVE, the one mandatory f32 compare)
      m  = c * fp16(s)        (GPSIMD)
      wb = m * sg             (DVE) -> resident fp16 weight wbt[k, o]
    f32 thresholds keep ternary rounding exact vs the reference (fp16
    thresholds measured at 1.1% rel err from flipped borderlines).
  - x streams in t-tiles of 128, accumulating in PSUM over 32 k-groups,
    3 output chunks (512/512/352) per core.
  - Prefix: dequant is emitted chunk-major; W1N t-tiles run group-outer
    in a window on chunk 0, pacing the PE right behind the dequant
    pipeline (~90% PE busy); W2N more run chunk 0 free; MIDN run chunks
    0-1; the rest run all chunks; skipped chunks catch up at the end.
    Per-tile PE cost is identical either way, so the reorder is free.
  - Per-core output [T, 1376] (t-major); host concatenates along O.
"""

import os

import numpy as np

import concourse.bass as bass
import concourse.mybir as mybir
import concourse.tile as tile
from concourse import bacc
from concourse.bass_utils import run_bass_kernel_spmd

P = 128
GROUP = 128
EPS = 1e-8

# Full problem shapes (hardcoded; harness calls kernel() with these).
FULL_B, FULL_S, FULL_K, FULL_O = 4, 2048, 4096, 11008
N_CORES = 8

DQB = 2          # k-groups per dequant block
CPAD = 512       # per-group chunk width in the padded wt/tp upload
W1N = 5          # paced warmup tiles (chunk 0, group-outer window)
W2N = 10         # free warmup tiles (chunk 0)
MIDN = 4         # tiles running chunks 0-1 only

LAST_RESULT = None  # BassKernelResults of the most recent run (for test.py)


def build_program(K, T, O_SHARD, mm_dt=mybir.dt.float16):
    """One SPMD program, identical on every core (data differs per core).

    DRAM tensors:
      xt  [T, K] fp16 ExternalInput -- x pre-packed on host (see pack_x) so
          that the per-t-tile load xt[tt*P+p, ko*G+t] = x[tt*P+t, ko*G+p]
          is one fully contiguous 1MB block (8KB per partition row)
      wt  [3, KO//DQB, P, DQB*CPAD] f32 ExternalInput -- weight shard,
          transposed to [k, o] and blocked per (chunk, group-pair); the
          352-wide chunk is zero-padded to CPAD
      tp  same shape/layout f32 -- thresholds s/2, replicated across the
          128 partitions (all k rows of a group share the group scale)
      sc  same shape/layout fp16 -- scales fp16(s), replicated likewise
      out [T, O_SHARD] f32 ExternalOutput
    """
    assert K % GROUP == 0 and T % P == 0
    KO = K // GROUP
    NB = KO // DQB  # dequant blocks per chunk
    n_ttiles = T // P
    OC = 512
    chunks = [(c0, min(OC, O_SHARD - c0)) for c0 in range(0, O_SHARD, OC)]
    n_chunks = len(chunks)
    W = DQB * CPAD

    nc = bacc.Bacc("TRN2", target_bir_lowering=False, debug=False)
    xt = nc.dram_tensor("xt", [T, K], mm_dt, kind="ExternalInput").ap()
    wt = nc.dram_tensor(
        "wt", [n_chunks, NB, P, W], mybir.dt.float32, kind="ExternalInput"
    ).ap()
    tp = nc.dram_tensor(
        "tp", [n_chunks, NB, P, W], mybir.dt.float32, kind="ExternalInput"
    ).ap()
    sc = nc.dram_tensor(
        "sc", [n_chunks, NB, P, W], mm_dt, kind="ExternalInput"
    ).ap()
    out = nc.dram_tensor(
        "out", [T, O_SHARD], mybir.dt.float32, kind="ExternalOutput"
    ).ap()

    with tile.TileContext(nc) as tc:
        with (
            tc.tile_pool(name="wres", bufs=1) as wres,
            tc.tile_pool(name="wload", bufs=3) as wload,
            tc.tile_pool(name="tpload", bufs=3) as tpload,
            tc.tile_pool(name="scload", bufs=2) as scload,
            tc.tile_pool(name="deq", bufs=2) as deq,
            tc.tile_pool(name="xin", bufs=6) as xin,
            tc.tile_pool(name="outp", bufs=2) as outp,
            tc.tile_pool(name="ps_a", bufs=4, space="PSUM") as ps_a,
            tc.tile_pool(name="ps_b", bufs=2, space="PSUM") as ps_b,
        ):
            # Resident dequantized weight, [k-part, group, o], one per chunk.
            wbt = [wres.tile([P, KO, csz], mm_dt, tag=f"wbt{ci}", name=f"wbt{ci}")
                   for ci, (c0, csz) in enumerate(chunks)]

            # ---------------- dequant of one (chunk, group-pair) ----------
            def dequant_block(ci, qb):
                c0, csz = chunks[ci]
                g0 = qb * DQB
                # w/tp/sc ride the ACT + GPSIMD DMA rings so they never
                # queue behind the much larger x stream on the SP ring.
                wt_t = wload.tile([P, W], mybir.dt.float32, tag="wt")
                nc.scalar.dma_start(wt_t, wt[ci, qb])
                tp_t = tpload.tile([P, W], mybir.dt.float32, tag="tp")
                nc.gpsimd.dma_start(tp_t, tp[ci, qb])
                sc_t = scload.tile([P, W], mm_dt, tag="sc")
                nc.gpsimd.dma_start(sc_t, sc[ci, qb])
                sg = deq.tile([P, W], mm_dt, tag="sg")
                nc.scalar.sign(sg, wt_t)
                nc.scalar.activation(            # aw = |w|, in place
                    wt_t, wt_t, mybir.ActivationFunctionType.Abs
                )
                c = deq.tile([P, W], mm_dt, tag="c")
                nc.vector.tensor_tensor(c, wt_t, tp_t, mybir.AluOpType.is_gt)
                m = deq.tile([P, W], mm_dt, tag="m")
                nc.gpsimd.tensor_tensor(m, c, sc_t, mybir.AluOpType.mult)
                # wb = m * sg in {-s, 0, +s}, straight into the resident tile
                m3 = m.rearrange("p (g c) -> p g c", c=CPAD)
                g3 = sg.rearrange("p (g c) -> p g c", c=CPAD)
                nc.vector.tensor_tensor(
                    wbt[ci][:, g0 : g0 + DQB, :],
                    m3[:, :, :csz],
                    g3[:, :, :csz],
                    mybir.AluOpType.mult,
                )

            # ---------------- matmul windows ----------------
            # host-packed: xt_r[tt, p, ko, t] = x[tt*P + t, ko*G + p]
            xt_r = xt.rearrange("(tt p) (ko t) -> tt p ko t", p=P, t=P)

            def alloc_ps(ci, tt, pool=None, tag=None):
                pool = pool or (ps_a if ci == 0 else ps_b)
                ps = pool.tile([P, OC], mybir.dt.float32,
                               tag=tag or f"mm{ci}", name=f"mm{ci}_{tt}")
                return ps[:, : chunks[ci][1]]

            def evac(tt, cis, pss):
                w0 = chunks[cis[0]][0]
                wid = sum(chunks[ci][1] for ci in cis)
                ot = outp.tile([P, O_SHARD], mybir.dt.float32, tag="ot",
                               name="ot")[:, :wid]
                for ci in cis:
                    c0, csz = chunks[ci]
                    nc.scalar.copy(ot[:, c0 - w0 : c0 - w0 + csz], pss[ci])
                nc.sync.dma_start(out[tt * P : tt * P + P, w0 : w0 + wid], ot)

            def mm_ttile(tt, cis):
                xb = xin.tile([P, KO, P], mm_dt, tag="xb", name=f"xb{tt}")
                nc.sync.dma_start(xb, xt_r[tt])
                pss = {ci: alloc_ps(ci, tt) for ci in cis}
                for ko in range(KO):
                    for ci in cis:
                        nc.tensor.matmul(
                            pss[ci],
                            lhsT=xb[:, ko, :],
                            rhs=wbt[ci][:, ko, :],
                            start=(ko == 0),
                            stop=(ko == KO - 1),
                        )
                evac(tt, cis, pss)

            # ---------------- emission order ----------------
            for ci in range(n_chunks):
                for qb in range(NB):
                    dequant_block(ci, qb)

            # W1: group-outer window of W1N tiles pacing the chunk-0 dequant
            xbs1, pss1 = [], []
            for tt in range(W1N):
                xb = xin.tile([P, KO, P], mm_dt, tag="xb", name=f"xb{tt}")
                nc.sync.dma_start(xb, xt_r[tt])
                xbs1.append(xb)
                pss1.append(alloc_ps(0, tt) if tt < 4 else
                            alloc_ps(0, tt, pool=ps_b, tag="mm1"))
            for ko in range(KO):
                for tt in range(W1N):
                    nc.tensor.matmul(
                        pss1[tt],
                        lhsT=xbs1[tt][:, ko, :],
                        rhs=wbt[0][:, ko, :],
                        start=(ko == 0),
                        stop=(ko == KO - 1),
                    )
            for tt in range(W1N):
                evac(tt, [0], {0: pss1[tt]})

            warm = list(range(W1N + W2N))
            mids = list(range(len(warm), len(warm) + MIDN))
            for tt in warm[W1N:]:
                mm_ttile(tt, [0])              # free-running chunk-0 warmup
            for tt in mids:
                mm_ttile(tt, [0, 1])           # chunk 2 still dequantizing
            for tt in range(len(warm) + MIDN, n_ttiles):
                mm_ttile(tt, [0, 1, 2])        # main loop
            for tt in warm:                    # catch-up
                mm_ttile(tt, [1, 2])
            for tt in mids:
                mm_ttile(tt, [2])

    nc.compile()
    return nc


def _run(nc, in_maps, trace=False):
    global LAST_RESULT
    res = run_bass_kernel_spmd(
        nc, in_maps, core_ids=list(range(len(in_maps))), trace=trace
    )
    LAST_RESULT = res
    return res


def pack_x(x2d):
    """[T, K] f32 -> fp16 packed: H[tt*P+p, ko*G+t] = x2d[tt*P+t, ko*G+p]."""
    T, K = x2d.shape
    x4 = x2d.reshape(T // P, P, K // GROUP, GROUP)  # [tt, t, ko, p]
    return np.ascontiguousarray(
        x4.transpose(0, 3, 2, 1).reshape(T, K).astype(np.float16)
    )


def pack_w(w_shard, chunks):
    """Weight shard [O_S, K] f32 -> (wt, tp, sc) blocked uploads.

    wt[ci, qb, p, gi*CPAD + u] = w_shard[c0 + u, (qb*DQB + gi)*GROUP + p]
    tp[ci, qb, p, gi*CPAD + u] = s[c0 + u, qb*DQB + gi] / 2  (f32)
    sc[...same...]             = fp16(s[c0 + u, qb*DQB + gi])
    (u < csz; zero-padded to CPAD).  s is the reference AbsMean scale.
    """
    O_S, K = w_shard.shape
    KO = K // GROUP
    NB = KO // DQB
    flat = w_shard.reshape(-1, GROUP).astype(np.float32)
    s = np.maximum(
        np.abs(flat).mean(axis=1, dtype=np.float32), np.float32(EPS)
    ).reshape(O_S, KO)                                    # [o, g]
    wT = np.ascontiguousarray(w_shard.T).reshape(KO, GROUP, O_S)  # [g, p, o]
    n_chunks = len(chunks)
    wt = np.zeros((n_chunks, NB, P, DQB * CPAD), dtype=np.float32)
    tp = np.zeros_like(wt)
    sc = np.zeros((n_chunks, NB, P, DQB * CPAD), dtype=np.float16)
    for ci, (c0, csz) in enumerate(chunks):
        blk = wT[:, :, c0 : c0 + csz].reshape(NB, DQB, P, csz)
        wt[ci].reshape(NB, P, DQB, CPAD)[..., :csz] = blk.transpose(0, 2, 1, 3)
        sblk = s.T[:, c0 : c0 + csz].reshape(NB, DQB, csz)
        tp[ci].reshape(NB, P, DQB, CPAD)[..., :csz] = (
            sblk[:, None, :, :] * np.float32(0.5)
        )
        sc[ci].reshape(NB, P, DQB, CPAD)[..., :csz] = sblk[
            :, None, :, :
        ].astype(np.float16)
    return wt, tp, sc


def kernel(x, weight):
    T = FULL_B * FULL_S
    K = FULL_K
    OS = FULL_O // N_CORES  # 1376
    chunks = [(c0, min(512, OS - c0)) for c0 in range(0, OS, 512)]
    x2d = pack_x(np.asarray(x, dtype=np.float32).reshape(T, K))
    w = np.asarray(weight, dtype=np.float32)

    nc = build_program(K, T, OS)
    in_maps = []
    for c in range(N_CORES):
        wt, tp, sc = pack_w(w[c * OS : (c + 1) * OS], chunks)
        in_maps.append({"xt": x2d, "wt": wt, "tp": tp, "sc": sc})
    trace = bool(os.environ.get("BASS_TRACE"))
    res = _run(nc, in_maps, trace=trace)
    full = np.concatenate(
        [res.results[c]["out"] for c in range(N_CORES)], axis=1
    )
    return np.ascontiguousarray(full.reshape(FULL_B, FULL_S, FULL_O))


# revision 5
# speedup vs baseline: 1.1683x; 1.0157x over previous
"""BitLinear (ternary-quantized linear) Trainium2 kernel, v4.

Computes: out = x @ dequant(weight).T where dequant is per-group(128)
AbsMean ternary quantization (w_q in {-1,0,+1} times per-group scale).

Strategy (8 NeuronCores, column-parallel / tensor-parallel):
  - weight [O=11008, K=4096] sharded by rows across 8 cores (1376 each).
  - x [B,S,K] -> [T=8192, K] replicated, pre-packed + pre-cast to fp16 on
    host (same RNE rounding the on-chip ACT cast would do) so the
    contraction dim lands on SBUF partitions and x DMA is halved.
  - The weight shard is uploaded pre-transposed to [k, o] blocks as two
    fp16 planes: u = |w| - s/2 (threshold margin) and v = sign(w)*fp16(s).
    The ternary decision and weight reconstruction run on-chip as two
    fp16 DVE passes with no PE/ACT/GPSIMD involvement:
      c  = (u > 0)          in {0,1}     (DVE tensor_scalar vs 0)
      wb = c * v            in {-s,0,+s} (DVE) -> resident fp16 wbt[k, o]
    Comparing u>0 in fp16 is sign-exact: fp16(u) can only lose the sign
    of u for |u| < 2^-25 (a handful of weights, each off by one ternary
    step - orders of magnitude inside the tolerance).  This matches the
    reference round(w/s) semantics including the strict-inequality
    borderline (|w| == s/2 -> 0).
  - x streams in t-tiles of 128, accumulating in PSUM over 32 k-groups,
    3 output chunks (512/512/352) per core.
  - Prefix: dequant blocks are emitted chunk-major; W1N t-tiles run
    group-outer in a window on chunk 0, pacing the PE right behind the
    dequant pipeline; W2N more run chunk 0 free; MIDN run chunks 0-1;
    the rest run all chunks; skipped chunks catch up at the end.
    Per-tile PE cost is identical either way, so the reorder is free.
  - Per-core output [T, 1376] (t-major); host concatenates along O.
"""

import os

import numpy as np

import concourse.bass as bass
import concourse.mybir as mybir
import concourse.tile as tile
from concourse import bacc
from concourse.bass_utils import run_bass_kernel_spmd

P = 128
GROUP = 128
EPS = 1e-8

# Full problem shapes (hardcoded; harness calls kernel() with these).
FULL_B, FULL_S, FULL_K, FULL_O = 4, 2048, 4096, 11008
N_CORES = 8

DQB = 4          # k-groups per dequant block
CPAD = 512       # per-group chunk width in the padded u/v upload
W1N = 5          # paced warmup tiles (chunk 0, group-outer window)
W2N = 8          # free warmup tiles (chunk 0)
MIDN = 2         # tiles running chunks 0-1 only

LAST_RESULT = None  # BassKernelResults of the most recent run (for test.py)


def build_program(K, T, O_SHARD, mm_dt=mybir.dt.float16):
    """One SPMD program, identical on every core (data differs per core).

    DRAM tensors:
      xt  [T, K] fp16 ExternalInput -- x pre-packed on host (see pack_x) so
          that the per-t-tile load xt[tt*P+p, ko*G+t] = x[tt*P+t, ko*G+p]
          is one fully contiguous 1MB block (8KB per partition row)
      u   [3, KO//DQB, P, DQB*CPAD] fp16 ExternalInput -- |w| - s/2,
          transposed to [k, o] and blocked per (chunk, group-quad); the
          352-wide chunk is zero-padded to CPAD
      v   same shape/layout fp16 -- sign(w) * fp16(s)
      out [T, O_SHARD] f32 ExternalOutput
    """
    assert K % GROUP == 0 and T % P == 0
    KO = K // GROUP
    NB = KO // DQB  # dequant blocks per chunk
    n_ttiles = T // P
    OC = 512
    chunks = [(c0, min(OC, O_SHARD - c0)) for c0 in range(0, O_SHARD, OC)]
    n_chunks = len(chunks)
    W = DQB * CPAD

    nc = bacc.Bacc("TRN2", target_bir_lowering=False, debug=False)
    xt = nc.dram_tensor("xt", [T, K], mm_dt, kind="ExternalInput").ap()
    u = nc.dram_tensor(
        "u", [n_chunks, NB, P, W], mm_dt, kind="ExternalInput"
    ).ap()
    v = nc.dram_tensor(
        "v", [n_chunks, NB, P, W], mm_dt, kind="ExternalInput"
    ).ap()
    out = nc.dram_tensor(
        "out", [T, O_SHARD], mybir.dt.float32, kind="ExternalOutput"
    ).ap()

    with tile.TileContext(nc) as tc:
        with (
            tc.tile_pool(name="wres", bufs=1) as wres,
            tc.tile_pool(name="uload", bufs=3) as uload,
            tc.tile_pool(name="vload", bufs=3) as vload,
            tc.tile_pool(name="deq", bufs=3) as deq,
            tc.tile_pool(name="xin", bufs=6) as xin,
            tc.tile_pool(name="outp", bufs=2) as outp,
            tc.tile_pool(name="ps_a", bufs=4, space="PSUM") as ps_a,
            tc.tile_pool(name="ps_b", bufs=2, space="PSUM") as ps_b,
        ):
            # Resident dequantized weight, [k-part, group, o], one per chunk.
            wbt = [wres.tile([P, KO, csz], mm_dt, tag=f"wbt{ci}", name=f"wbt{ci}")
                   for ci, (c0, csz) in enumerate(chunks)]

            # ---------------- dequant of one (chunk, group-quad) ----------
            def dequant_block(ci, qb):
                c0, csz = chunks[ci]
                g0 = qb * DQB
                # u/v ride the ACT + GPSIMD DMA rings so they never queue
                # behind the much larger x stream on the SP ring.
                u_t = uload.tile([P, W], mm_dt, tag="u")
                nc.scalar.dma_start(u_t, u[ci, qb])
                v_t = vload.tile([P, W], mm_dt, tag="v")
                nc.gpsimd.dma_start(v_t, v[ci, qb])
                c = deq.tile([P, W], mm_dt, tag="c")
                nc.vector.tensor_scalar(
                    c, u_t, 0.0, None, mybir.AluOpType.is_gt
                )
                c3 = c.rearrange("p (g c) -> p g c", c=CPAD)
                v3 = v_t.rearrange("p (g c) -> p g c", c=CPAD)
                nc.vector.tensor_tensor(
                    wbt[ci][:, g0 : g0 + DQB, :],
                    c3[:, :, :csz],
                    v3[:, :, :csz],
                    mybir.AluOpType.mult,
                )

            # ---------------- matmul windows ----------------
            # host-packed: xt_r[tt, p, ko, t] = x[tt*P + t, ko*G + p]
            xt_r = xt.rearrange("(tt p) (ko t) -> tt p ko t", p=P, t=P)

            def alloc_ps(ci, tt, pool=None, tag=None):
                pool = pool or (ps_a if ci == 0 else ps_b)
                ps = pool.tile([P, OC], mybir.dt.float32,
                               tag=tag or f"mm{ci}", name=f"mm{ci}_{tt}")
                return ps[:, : chunks[ci][1]]

            def evac(tt, cis, pss):
                w0 = chunks[cis[0]][0]
                wid = sum(chunks[ci][1] for ci in cis)
                ot = outp.tile([P, O_SHARD], mybir.dt.float32, tag="ot",
                               name="ot")[:, :wid]
                for ci in cis:
                    c0, csz = chunks[ci]
                    nc.scalar.copy(ot[:, c0 - w0 : c0 - w0 + csz], pss[ci])
                nc.sync.dma_start(out[tt * P : tt * P + P, w0 : w0 + wid], ot)

            def mm_ttile(tt, cis):
                xb = xin.tile([P, KO, P], mm_dt, tag="xb", name=f"xb{tt}")
                nc.sync.dma_start(xb, xt_r[tt])
                pss = {ci: alloc_ps(ci, tt) for ci in cis}
                for ko in range(KO):
                    for ci in cis:
                        nc.tensor.matmul(
                            pss[ci],
                            lhsT=xb[:, ko, :],
                            rhs=wbt[ci][:, ko, :],
                            start=(ko == 0),
                            stop=(ko == KO - 1),
                        )
                evac(tt, cis, pss)

            # ---------------- emission order ----------------
            for ci in range(n_chunks):
                for qb in range(NB):
                    dequant_block(ci, qb)

            # W1: group-outer window of W1N tiles pacing the chunk-0 dequant
            xbs1, pss1 = [], []
            for tt in range(W1N):
                xb = xin.tile([P, KO, P], mm_dt, tag="xb", name=f"xb{tt}")
                nc.sync.dma_start(xb, xt_r[tt])
                xbs1.append(xb)
                pss1.append(alloc_ps(0, tt) if tt < 4 else
                            alloc_ps(0, tt, pool=ps_b, tag="mm1"))
            for ko in range(KO):
                for tt in range(W1N):
                    nc.tensor.matmul(
                        pss1[tt],
                        lhsT=xbs1[tt][:, ko, :],
                        rhs=wbt[0][:, ko, :],
                        start=(ko == 0),
                        stop=(ko == KO - 1),
                    )
            for tt in range(W1N):
                evac(tt, [0], {0: pss1[tt]})

            warm = list(range(W1N + W2N))
            mids = list(range(len(warm), len(warm) + MIDN))
            for tt in warm[W1N:]:
                mm_ttile(tt, [0])              # free-running chunk-0 warmup
            for tt in mids:
                mm_ttile(tt, [0, 1])           # chunk 2 still dequantizing
            for tt in range(len(warm) + MIDN, n_ttiles):
                mm_ttile(tt, [0, 1, 2])        # main loop
            for tt in warm:                    # catch-up
                mm_ttile(tt, [1, 2])
            for tt in mids:
                mm_ttile(tt, [2])

    nc.compile()
    return nc


def _run(nc, in_maps, trace=False):
    global LAST_RESULT
    res = run_bass_kernel_spmd(
        nc, in_maps, core_ids=list(range(len(in_maps))), trace=trace
    )
    LAST_RESULT = res
    return res


def pack_x(x2d):
    """[T, K] f32 -> fp16 packed: H[tt*P+p, ko*G+t] = x2d[tt*P+t, ko*G+p]."""
    T, K = x2d.shape
    x4 = x2d.reshape(T // P, P, K // GROUP, GROUP)  # [tt, t, ko, p]
    return np.ascontiguousarray(
        x4.transpose(0, 3, 2, 1).reshape(T, K).astype(np.float16)
    )


def pack_w(w_shard, chunks):
    """Weight shard [O_S, K] f32 -> (u, v) fp16 blocked uploads.

    u[ci, qb, p, gi*CPAD + j] = fp16(|w[o,k]| - s[o,g]/2)   (sign-exact)
    v[ci, qb, p, gi*CPAD + j] = sign(w[o,k]) * fp16(s[o,g])
    with o = c0 + j, g = qb*DQB + gi, k = g*GROUP + p;
    j < csz, zero-padded to CPAD.  s is the reference AbsMean scale.
    """
    O_S, K = w_shard.shape
    KO = K // GROUP
    NB = KO // DQB
    w32 = w_shard.astype(np.float32)
    flat = w32.reshape(-1, GROUP)
    s = np.maximum(
        np.abs(flat).mean(axis=1, dtype=np.float32), np.float32(EPS)
    ).reshape(O_S, KO)                                    # [o, g]
    s16 = s.astype(np.float16)
    su = np.repeat(s * np.float32(0.5), GROUP, axis=1)    # [o, k] thresholds
    sv = np.repeat(s16.astype(np.float32), GROUP, axis=1)
    u_full = (np.abs(w32) - su).astype(np.float16)        # [o, k]
    v_full = (np.sign(w32) * sv).astype(np.float16)
    n_chunks = len(chunks)
    u = np.zeros((n_chunks, NB, P, DQB * CPAD), dtype=np.float16)
    v = np.zeros_like(u)
    for ci, (c0, csz) in enumerate(chunks):
        for src, dst in ((u_full, u), (v_full, v)):
            blk = np.ascontiguousarray(src[c0 : c0 + csz].T).reshape(
                KO, GROUP, csz
            )                                             # [g, p, o]
            dst[ci].reshape(NB, P, DQB, CPAD)[..., :csz] = blk.reshape(
                NB, DQB, P, csz
            ).transpose(0, 2, 1, 3)
    return u, v


def kernel(x, weight):
    T = FULL_B * FULL_S
    K = FULL_K
    OS = FULL_O // N_CORES  # 1376
    chunks = [(c0, min(512, OS - c0)) for c0 in range(0, OS, 512)]
    x2d = pack_x(np.asarray(x, dtype=np.float32).reshape(T, K))
    w = np.asarray(weight, dtype=np.float32)

    nc = build_program(K, T, OS)
    in_maps = []
    for c in range(N_CORES):
        uu, vv = pack_w(w[c * OS : (c + 1) * OS], chunks)
        in_maps.append({"xt": x2d, "u": uu, "v": vv})
    trace = bool(os.environ.get("BASS_TRACE"))
    res = _run(nc, in_maps, trace=trace)
    full = np.concatenate(
        [res.results[c]["out"] for c in range(N_CORES)], axis=1
    )
    return np.ascontiguousarray(full.reshape(FULL_B, FULL_S, FULL_O))


# revision 6
# speedup vs baseline: 1.2786x; 1.0945x over previous
"""BitLinear (ternary-quantized linear) Trainium2 kernel, v4.

Computes: out = x @ dequant(weight).T where dequant is per-group(128)
AbsMean ternary quantization (w_q in {-1,0,+1} times per-group scale).

Strategy (8 NeuronCores, column-parallel / tensor-parallel):
  - weight [O=11008, K=4096] sharded by rows across 8 cores (1376 each).
  - x [B,S,K] -> [T=8192, K] replicated, pre-packed + pre-cast to fp16 on
    host (same RNE rounding the on-chip ACT cast would do) so the
    contraction dim lands on SBUF partitions and x DMA is halved.
  - The weight shard is uploaded pre-transposed to [k, o] blocks as two
    fp16 planes: u = |w| - s/2 (threshold margin) and v = sign(w)*fp16(s).
    The ternary decision and weight reconstruction run on-chip as two
    fp16 DVE passes with no PE/ACT/GPSIMD involvement:
      c  = (u > 0)          in {0,1}     (DVE tensor_scalar vs 0)
      wb = c * v            in {-s,0,+s} (DVE) -> resident fp16 wbt[k, o]
    Comparing u>0 in fp16 is sign-exact: fp16(u) can only lose the sign
    of u for |u| < 2^-25 (a handful of weights, each off by one ternary
    step - orders of magnitude inside the tolerance).  This matches the
    reference round(w/s) semantics including the strict-inequality
    borderline (|w| == s/2 -> 0).
  - x streams in t-tiles of 128, accumulating in PSUM over 32 k-groups,
    3 output chunks (512/512/352) per core.
  - Prefix: dequant blocks are emitted chunk-major; W1N t-tiles run
    group-outer in a window on chunk 0, pacing the PE right behind the
    dequant pipeline; W2N more run chunk 0 free; MIDN run chunks 0-1;
    the rest run all chunks; skipped chunks catch up at the end.
    Per-tile PE cost is identical either way, so the reorder is free.
  - Per-core output [T, 1376] (t-major); host concatenates along O.
"""

import os

import numpy as np

import concourse.bass as bass
import concourse.mybir as mybir
import concourse.tile as tile
from concourse import bacc
from concourse.bass_utils import run_bass_kernel_spmd

P = 128
GROUP = 128
EPS = 1e-8

# Full problem shapes (hardcoded; harness calls kernel() with these).
FULL_B, FULL_S, FULL_K, FULL_O = 4, 2048, 4096, 11008
N_CORES = 8

DQB = 4          # k-groups per dequant block
CPAD = 512       # per-group chunk width in the padded u/v upload
W1N = 6          # paced warmup tiles (chunk 0, group-outer window)
W2N = 4          # free warmup tiles (chunk 0)
MIDN = 3         # tiles running chunks 0-1 only

LAST_RESULT = None  # BassKernelResults of the most recent run (for test.py)


def build_program(K, T, O_SHARD, mm_dt=mybir.dt.float16):
    """One SPMD program, identical on every core (data differs per core).

    DRAM tensors:
      xt  [T, K] fp16 ExternalInput -- x pre-packed on host (see pack_x) so
          that the per-t-tile load xt[tt*P+p, ko*G+t] = x[tt*P+t, ko*G+p]
          is one fully contiguous 1MB block (8KB per partition row)
      u   [3, KO//DQB, P, DQB*CPAD] fp16 ExternalInput -- |w| - s/2,
          transposed to [k, o] and blocked per (chunk, group-quad); the
          352-wide chunk is zero-padded to CPAD
      v   same shape/layout fp16 -- sign(w) * fp16(s)
      out [T, O_SHARD] f32 ExternalOutput
    """
    assert K % GROUP == 0 and T % P == 0
    KO = K // GROUP
    NB = KO // DQB  # dequant blocks per chunk
    n_ttiles = T // P
    OC = 512
    chunks = [(c0, min(OC, O_SHARD - c0)) for c0 in range(0, O_SHARD, OC)]
    n_chunks = len(chunks)
    W = DQB * CPAD

    nc = bacc.Bacc("TRN2", target_bir_lowering=False, debug=False)
    xt = nc.dram_tensor("xt", [T, K], mm_dt, kind="ExternalInput").ap()
    u = nc.dram_tensor(
        "u", [n_chunks, NB, P, W], mybir.dt.bfloat16, kind="ExternalInput"
    ).ap()
    v = nc.dram_tensor(
        "v", [n_chunks, NB, P, W], mm_dt, kind="ExternalInput"
    ).ap()
    out = nc.dram_tensor(
        "out", [T, O_SHARD], mybir.dt.float32, kind="ExternalOutput"
    ).ap()

    with tile.TileContext(nc) as tc:
        with (
            tc.tile_pool(name="wres", bufs=1) as wres,
            tc.tile_pool(name="uload", bufs=2) as uload,
            tc.tile_pool(name="vload", bufs=2) as vload,
            tc.tile_pool(name="deq", bufs=2) as deq,
            tc.tile_pool(name="xin", bufs=8) as xin,
            tc.tile_pool(name="outp", bufs=2) as outp,
            tc.tile_pool(name="ps_a", bufs=4, space="PSUM") as ps_a,
            tc.tile_pool(name="ps_b", bufs=2, space="PSUM") as ps_b,
        ):
            # Resident dequantized weight, [k-part, group, o], one per chunk.
            wbt = [wres.tile([P, KO, csz], mm_dt, tag=f"wbt{ci}", name=f"wbt{ci}")
                   for ci, (c0, csz) in enumerate(chunks)]

            # ---------------- dequant of one (chunk, group-quad) ----------
            def dequant_block(ci, qb):
                c0, csz = chunks[ci]
                g0 = qb * DQB
                # u/v ride the ACT + GPSIMD DMA rings so they never queue
                # behind the much larger x stream on the SP ring.
                u_t = uload.tile([P, W], mybir.dt.bfloat16, tag="u")
                nc.scalar.dma_start(u_t, u[ci, qb])
                v_t = vload.tile([P, W], mm_dt, tag="v")
                nc.gpsimd.dma_start(v_t, v[ci, qb])
                c = deq.tile([P, W], mm_dt, tag="c")
                nc.vector.tensor_scalar(
                    c, u_t, 0.0, None, mybir.AluOpType.is_gt
                )
                c3 = c.rearrange("p (g c) -> p g c", c=CPAD)
                v3 = v_t.rearrange("p (g c) -> p g c", c=CPAD)
                nc.vector.tensor_tensor(
                    wbt[ci][:, g0 : g0 + DQB, :],
                    c3[:, :, :csz],
                    v3[:, :, :csz],
                    mybir.AluOpType.mult,
                )

            # ---------------- matmul windows ----------------
            # host-packed: xt_r[tt, p, ko, t] = x[tt*P + t, ko*G + p]
            xt_r = xt.rearrange("(tt p) (ko t) -> tt p ko t", p=P, t=P)

            def alloc_ps(ci, tt, pool=None, tag=None):
                pool = pool or (ps_a if ci == 0 else ps_b)
                ps = pool.tile([P, OC], mybir.dt.float32,
                               tag=tag or f"mm{ci}", name=f"mm{ci}_{tt}")
                return ps[:, : chunks[ci][1]]

            def evac(tt, cis, pss):
                w0 = chunks[cis[0]][0]
                wid = sum(chunks[ci][1] for ci in cis)
                ot = outp.tile([P, O_SHARD], mybir.dt.float32, tag="ot",
                               name="ot")[:, :wid]
                for ci in cis:
                    c0, csz = chunks[ci]
                    nc.scalar.copy(ot[:, c0 - w0 : c0 - w0 + csz], pss[ci])
                nc.sync.dma_start(out[tt * P : tt * P + P, w0 : w0 + wid], ot)

            def mm_ttile(tt, cis, xring=None):
                xb = xin.tile([P, KO, P], mm_dt, tag="xb", name=f"xb{tt}")
                (xring or nc.sync).dma_start(xb, xt_r[tt])
                pss = {ci: alloc_ps(ci, tt) for ci in cis}
                for ko in range(KO):
                    for ci in cis:
                        nc.tensor.matmul(
                            pss[ci],
                            lhsT=xb[:, ko, :],
                            rhs=wbt[ci][:, ko, :],
                            start=(ko == 0),
                            stop=(ko == KO - 1),
                        )
                evac(tt, cis, pss)

            # ---------------- emission order ----------------
            for ci in range(n_chunks):
                for qb in range(NB):
                    dequant_block(ci, qb)

            # W1: group-outer window of W1N tiles pacing the chunk-0 dequant
            xbs1, pss1 = [], []
            for tt in range(W1N):
                xb = xin.tile([P, KO, P], mm_dt, tag="xb", name=f"xb{tt}")
                nc.sync.dma_start(xb, xt_r[tt])
                xbs1.append(xb)
                pss1.append(alloc_ps(0, tt) if tt < 4 else
                            alloc_ps(0, tt, pool=ps_b,
                                     tag="mm1" if tt == 4 else "mm2"))
            for ko in range(KO):
                for tt in range(W1N):
                    nc.tensor.matmul(
                        pss1[tt],
                        lhsT=xbs1[tt][:, ko, :],
                        rhs=wbt[0][:, ko, :],
                        start=(ko == 0),
                        stop=(ko == KO - 1),
                    )
            for tt in range(W1N):
                evac(tt, [0], {0: pss1[tt]})

            warm = list(range(W1N + W2N))
            mids = list(range(len(warm), len(warm) + MIDN))
            for tt in warm[W1N:]:
                mm_ttile(tt, [0])              # free-running chunk-0 warmup
            for tt in mids:
                mm_ttile(tt, [0, 1])           # chunk 2 still dequantizing
            for tt in range(len(warm) + MIDN, n_ttiles):
                # alternate x loads between the SP and ACT rings
                mm_ttile(tt, [0, 1, 2],
                         xring=nc.scalar if tt % 2 else nc.sync)
            for tt in warm:                    # catch-up
                mm_ttile(tt, [1, 2])
            for tt in mids:
                mm_ttile(tt, [2])

    nc.compile()
    return nc


def _run(nc, in_maps, trace=False):
    global LAST_RESULT
    res = run_bass_kernel_spmd(
        nc, in_maps, core_ids=list(range(len(in_maps))), trace=trace
    )
    LAST_RESULT = res
    return res


def pack_x(x2d):
    """[T, K] f32 -> fp16 packed: H[tt*P+p, ko*G+t] = x2d[tt*P+t, ko*G+p]."""
    T, K = x2d.shape
    x4 = x2d.reshape(T // P, P, K // GROUP, GROUP)  # [tt, t, ko, p]
    return np.ascontiguousarray(
        x4.transpose(0, 3, 2, 1).reshape(T, K).astype(np.float16)
    )


def pack_w(w_shard, chunks):
    """Weight shard [O_S, K] f32 -> (u, v) fp16 blocked uploads.

    u[ci, qb, p, gi*CPAD + j] = fp16(|w[o,k]| - s[o,g]/2)   (sign-exact)
    v[ci, qb, p, gi*CPAD + j] = sign(w[o,k]) * fp16(s[o,g])
    with o = c0 + j, g = qb*DQB + gi, k = g*GROUP + p;
    j < csz, zero-padded to CPAD.  s is the reference AbsMean scale.
    """
    O_S, K = w_shard.shape
    KO = K // GROUP
    NB = KO // DQB
    w32 = w_shard.astype(np.float32)
    flat = w32.reshape(-1, GROUP)
    s = np.maximum(
        np.abs(flat).mean(axis=1, dtype=np.float32), np.float32(EPS)
    ).reshape(O_S, KO)                                    # [o, g]
    s16 = s.astype(np.float16)
    su = np.repeat(s * np.float32(0.5), GROUP, axis=1)    # [o, k] thresholds
    sv = np.repeat(s16.astype(np.float32), GROUP, axis=1)
    import ml_dtypes
    u_full = (np.abs(w32) - su).astype(ml_dtypes.bfloat16)  # [o, k]
    v_full = (np.sign(w32) * sv).astype(np.float16)
    n_chunks = len(chunks)
    import ml_dtypes
    u = np.zeros((n_chunks, NB, P, DQB * CPAD), dtype=ml_dtypes.bfloat16)
    v = np.zeros((n_chunks, NB, P, DQB * CPAD), dtype=np.float16)
    for ci, (c0, csz) in enumerate(chunks):
        for src, dst in ((u_full, u), (v_full, v)):
            blk = np.ascontiguousarray(src[c0 : c0 + csz].T).reshape(
                KO, GROUP, csz
            )                                             # [g, p, o]
            dst[ci].reshape(NB, P, DQB, CPAD)[..., :csz] = blk.reshape(
                NB, DQB, P, csz
            ).transpose(0, 2, 1, 3)
    return u, v


def kernel(x, weight):
    T = FULL_B * FULL_S
    K = FULL_K
    OS = FULL_O // N_CORES  # 1376
    chunks = [(c0, min(512, OS - c0)) for c0 in range(0, OS, 512)]
    x2d = pack_x(np.asarray(x, dtype=np.float32).reshape(T, K))
    w = np.asarray(weight, dtype=np.float32)

    nc = build_program(K, T, OS)
    in_maps = []
    for c in range(N_CORES):
        uu, vv = pack_w(w[c * OS : (c + 1) * OS], chunks)
        in_maps.append({"xt": x2d, "u": uu, "v": vv})
    trace = bool(os.environ.get("BASS_TRACE"))
    res = _run(nc, in_maps, trace=trace)
    full = np.concatenate(
        [res.results[c]["out"] for c in range(N_CORES)], axis=1
    )
    return np.ascontiguousarray(full.reshape(FULL_B, FULL_S, FULL_O))
